# revision 55
# baseline (speedup 1.0000x reference)
"""Trainium2 Bass kernel for nn_DEC_26139170963600 (vq_codebook).

Reference computation:
  4x strided conv1d (stride 2, VALID) with LeakyReLU(0.1) between layers,
  flatten -> soft VQ assignment over 64 centers:
      d2 = ||z||^2 + ||c||^2 - 2 z.c
      q  = (1/(1+d2)) row-normalized            (alpha=1 -> exponent is 1)

Sharding: data-parallel over batch N=256 across 8 cores (32 samples/core).
Weights / centers replicated. No cross-device communication.

Per-core kernel design (fp8 DoubleRow conv stack):
  - x and all conv weights quantized host-side to fp8e4 (TRN E4M3, max 240).
    Weights are pre-scaled by a per-layer power of two (up to ~224 max mag)
    to clear e4m3 subnormals; the inverse scale rides the PSUM eviction.
  - conv layer = K/2 tap-pair matmuls in MatmulPerfMode.DoubleRow, which
    virtualizes the PE to a 256-deep contraction (2 fp8 weights/cell):
        out[o, l] += sum_i W[o,:,k+i]^T . h[:, 2l+k+i],  i in {0,1}
    lhsT = (128, 2, 128) tap-pair slice of the packed weights; rhs is the
    contiguous slice h[k : k+2*Lout] rearranged "(l two) -> two l" (the
    stride-2 conv makes tap pairs adjacent in memory). Odd K padded with a
    zero tap (conv1 15->16, conv3 7->8; h2 rows are stride-248-padded so
    the zero tap's read stays in-bounds).
  - PSUM eviction is TWO ops (was 3): ACT A = ps*2^-k + b (PSUM->SBUF
    bf16), then the whole LeakyReLU as ONE DVE scalar_tensor_tensor:
        h' = (A * 0.1) max A -> fp8   (exact lrelu since 0.1 < 1)
    This halves the old DVE+Pool eviction backlog that stalled the PE.
  - HW-verified op constraints (all-8-core probes; do not regress):
      * scalar_tensor_tensor is DVE-only (fails Pool's neuronxcc ISA check)
      * DVE tensor_scalar divide fails the ISA check
      * ACT Prelu hard-faults the device when >=4 cores run it (probe-
        verified in a previous session); the conv4 z8/Square-on-ACT
        variant and 2-sample conv blocks showed the same fault class at
        kernel scale - all avoided here. fp16 matmuls hard-fault too.
  - conv4 evicts bf16 z (no lrelu); distance stays bf16: 59 matmuls
    accumulate -2 z.c into PSUM (32n x 64j); ||z||^2 via per-group DVE
    square+reduce overlapped with conv4, then an fp32 matmul against a
    ones column; 1 + ||c||^2 is a host-precomputed (32,64) fp32 tile.
  - q = reciprocal(1+d2) row-normalized on DVE, DMA out as fp32.
  - PE pre-warm: dummy matmuls during the w1/x DMA lead-in so HAM
    un-throttles (0.65 -> 2.4 GHz) before real conv work arrives.

Measured (8 axon trn2 cores): max rel err ~2.3e-3 vs fp32 reference.
TimelineSim exec time: see test.py (original baseline was 63209 ns).
"""

import os
import sys

import numpy as np
import ml_dtypes

for _p in ("/opt/trn_rl_repo",):
    if _p not in sys.path and os.path.isdir(_p):
        sys.path.insert(0, _p)

import concourse.bacc as bacc  # noqa: E402
import concourse.mybir as mybir  # noqa: E402
import concourse.tile as tile  # noqa: E402
from concourse import bass_utils  # noqa: E402

F8 = mybir.dt.float8e4
HDT = mybir.dt.bfloat16
F32 = mybir.dt.float32
AF = mybir.ActivationFunctionType
OP = mybir.AluOpType
DR = mybir.MatmulPerfMode.DoubleRow

N_CORES = 8
NS = 32          # samples per core
C = 128          # channels
KCENT = 64       # number of centers
LFIN = 59        # final length
D = C * LFIN     # 7552

# (K_real, K_padded, L_in_row_stride, L_out, L_out_row_stride, G samples/mm)
# conv2's output rows carry 1 pad element (248) so conv3's zero tap 7 reads
# in-bounds; the pad is memset once.
CFG = [
    (15, 16, 1024, 505, 505, 1),
    (12, 12, 505, 247, 248, 2),
    (7, 8, 248, 121, 121, 4),
    (4, 4, 121, 59, 59, 8),
]

N_WARM = 29  # PE pre-warm dummy matmuls


def _schedule():
    """Block emission order (identical to the HW-proven baseline weave):
    four 8-sample conv streams s0-s3 (conv1 blocks = 2 samples, conv2 = 4,
    conv3/4 = 8) and two 16-sample distance regions; each stage's eviction
    latency hides under another stream's PE work."""
    seq = [("c", 0, p) for p in range(16)]
    seq += [("c", 1, p) for p in range(8)]
    seq += [("c", 2, p) for p in range(4)]
    seq += [("c", 3, p) for p in range(4)]
    seq += [("d", 0), ("q", 0), ("d", 1), ("q", 1)]
    return seq


_BUILt = {}


def _build_program(n_repeat=1):
    """Build + compile the per-core Bass program (same program on all cores)."""
    nc = bacc.Bacc("TRN2", target_bir_lowering=False, debug=False)

    # ---- DRAM I/O ----
    x_d = nc.dram_tensor("x", (C, NS, 1024), F8, kind="ExternalInput")
    w_d = [
        nc.dram_tensor(f"w{i+1}", (C, CFG[i][1] * C), F8, kind="ExternalInput")
        for i in range(4)
    ]
    # bias/scale pack: cols 0-3 = b1..b4; cols 4-7 = 2^-k1..2^-k4;
    # cols 8-71 = ones (zn mm rhs); cols 72-135 = row0-only 1+||c||^2
    # (cn mm rhs); cols 136-167 = e0 (partition-0 ones, cn mm lhsT)
    bp_d = nc.dram_tensor("bp", (C, 170), F32, kind="ExternalInput")
    # fp8 centers: cr8[c, l*64+j] = (-2/16)*centers[j, c*59+l], chunk 59 = 0
    cr_d = nc.dram_tensor("cr", (C, 60 * KCENT), F8, kind="ExternalInput")
    q_d = nc.dram_tensor("q", (NS, KCENT), F32, kind="ExternalOutput")

    with tile.TileContext(nc) as tc:
        with (
            tc.tile_pool(name="consts", bufs=1) as cpool,
            tc.tile_pool(name="xp", bufs=8) as xpool,
            tc.tile_pool(name="hp", bufs=1) as hpool,
            tc.tile_pool(name="ap", bufs=4) as apool,
            tc.tile_pool(name="tp", bufs=4) as tpool,
            tc.tile_pool(name="small", bufs=1) as mpool,
            tc.tile_pool(name="psA", bufs=3, space="PSUM") as psA,
            tc.tile_pool(name="psD", bufs=2, space="PSUM") as psD,
        ):
            wt = [
                cpool.tile([C, CFG[i][1] * C], F8, tag=f"w{i}", name=f"wt{i}")
                for i in range(4)
            ]
            bp = cpool.tile([C, 170], F32, tag="bp")
            cr = cpool.tile([C, 60 * KCENT], F8, tag="cr")

            for _rep in range(n_repeat):
                _body_once(nc, tc, x_d, q_d, w_d, bp_d, cr_d, wt, bp,
                           cr, xpool, hpool, apool, tpool, mpool,
                           psA, psD, load_consts=(_rep == 0))

    nc.compile()
    return nc


def _body_once(nc, tc, x_d, q_d, w_d, bp_d, cr_d, wt, bp, cr,
               xpool, hpool, apool, tpool, mpool, psA, psD,
               load_consts=True):
            # ---- One prioritized DMA stream on the SP ring (the HWDGE is
            # a single shared resource): w1 taps 0-7, x samples 0/1, w1
            # taps 8-15, bp, then the remaining x chunks with w2-4 and cr
            # slotted no earlier than their first use needs ----
            w1v = w_d[0].ap().rearrange("p (k o) -> p k o", o=C)
            wt1v = wt[0][:].rearrange("p (k o) -> p k o", o=C)
            if load_consts:
                nc.sync.dma_start(wt1v[:, 0:8, :], w1v[:, 0:8, :])
            x3d = x_d.ap()  # (C, NS, 1024)
            xch = []
            xt = xpool.tile([C, 2 * 1024], F8, tag="x", name="xch0")
            xt3 = xt[:].rearrange("p (a b) -> p a b", a=2)
            nc.sync.dma_start(xt3[:, 0:1, :], x3d[:, 0:1, :])
            nc.sync.dma_start(xt3[:, 1:2, :], x3d[:, 1:2, :])
            if load_consts:
                nc.sync.dma_start(wt1v[:, 8:16, :], w1v[:, 8:16, :])
                nc.sync.dma_start(bp[:], bp_d.ap())
            xch.append(xt)
            for g in range(1, 16):
                t = xpool.tile([C, 2 * 1024], F8, tag="x", name=f"xch{g}")
                src = x3d[:, 2 * g : 2 * g + 2, :].rearrange("p a b -> p (a b)")
                nc.sync.dma_start(t[:], src)
                xch.append(t)
                if load_consts and g == 3:
                    nc.sync.dma_start(wt[1][:], w_d[1].ap())
                if load_consts and g == 5:
                    nc.sync.dma_start(wt[2][:], w_d[2].ap())
                    nc.sync.dma_start(wt[3][:], w_d[3].ap())
                if load_consts and g == 8:
                    nc.sync.dma_start(cr[:], cr_d.ap())

                # ---- PE pre-warm: HAM un-throttles (0.65 -> 2.4 GHz) after
                # ~3us of sustained activity; burn the w1/x0 DMA lead-in on
                # dummy matmuls over a zeroed scratch so conv1 starts warm ----
                wsrc = tpool.tile([1, 128], HDT, tag="warm", name="warm")
                nc.gpsimd.memset(wsrc[:], 0.0)
                wps = psA.tile([C, 128], F32, tag="ps", name="warmps")
                for _w in range(N_WARM):
                    nc.tensor.matmul(
                        wps[:], wsrc[:], wsrc[:],
                        start=(_w == 0), stop=(_w == N_WARM - 1)
                    )

            # ---- shared tiles for the conv stack / distance tail ----
            h_tiles = []
            for li, (K, Kp, Lin, Lout, Lrow, G) in enumerate(CFG):
                if li == 3:
                    hdst = hpool.tile([C, NS * Lout], HDT, tag=f"h{li}")
                else:
                    hdst = hpool.tile([C, NS * Lrow], F8, tag=f"h{li}")
                    if Lrow > Lout:
                        # zero the per-sample pad so the zero tap's
                        # in-bounds read never multiplies NaN garbage
                        padv = hdst[:].rearrange("p (n l) -> p n l", n=NS)
                        nc.gpsimd.memset(padv[:, :, Lout:Lrow], 0.0)
                h_tiles.append(hdst)
            zsq = hpool.tile([C, NS * LFIN], F32, tag="zsq", name="zsq")
            part = mpool.tile([C, NS], F32, tag="part", name="part")
            z8 = hpool.tile([C, 60 * NS], F8, tag="z8", name="z8")
            nc.gpsimd.memset(z8[:, LFIN * NS : 60 * NS], 0.0)
            z84 = z8[:].rearrange("p (l n) -> p l n", n=NS)
            cr3 = cr[:].rearrange("p (l j) -> p l j", j=KCENT)
            dps = [None] * 2

            def conv_block(li, pr):
                """One PSUM block: conv1-3 = two G-sample halves sharing a
                2-bank tile; conv4 = one 8-sample group on half a tile."""
                K, Kp, Lin, Lout, Lrow, G = CFG[li]
                hdst = h_tiles[li]
                hdst3 = hdst[:].rearrange("p (n l) -> p n l", n=NS)
                if li > 0:
                    hsrc3 = h_tiles[li - 1][:].rearrange("p (n l) -> p n l", n=NS)
                nhalf = 1 if li == 3 else 2
                gp = pr * nhalf * G
                ps = psA.tile([C, 1024], F32, tag="ps")
                for half in range(nhalf):
                    g0 = gp + half * G
                    pslice = ps[:, half * 512 : half * 512 + G * Lout]
                    for kp in range(0, Kp, 2):
                        lhsT = wt[li][:, kp * C : (kp + 2) * C].rearrange(
                            "p (two o) -> p two o", two=2
                        )
                        if li == 0:
                            x3 = xch[g0 // 2][:].rearrange("p (a b) -> p a b", a=2)
                            rhs = x3[
                                :, g0 % 2 : g0 % 2 + 1, kp : kp + 2 * Lout
                            ].rearrange("p n (l two) -> p two n l", two=2)
                        else:
                            rhs = hsrc3[
                                :, g0 : g0 + G, kp : kp + 2 * Lout
                            ].rearrange("p n (l two) -> p two n l", two=2)
                        nc.tensor.matmul(
                            pslice, lhsT, rhs,
                            start=(kp == 0), stop=(kp == Kp - 2),
                            perf_mode=DR,
                        )
                bias = bp[:, li : li + 1]
                scale = bp[:, 4 + li : 5 + li]
                ng = nhalf * G
                E = ng * Lout
                psv = (
                    ps[:, 0 : G * Lout]
                    if li == 3
                    else ps[:].rearrange("p (g l) -> p g l", g=2)[:, :, 0 : G * Lout]
                )
                if li < 3:
                    # eviction v2: ACT A = ps*2^-k + b, then lrelu in ONE
                    # DVE op: h = (0.1*A) max A -> fp8
                    dsl = hdst3[:, gp : gp + ng, 0:Lout]
                    A = apool.tile([C, E], HDT, tag="A")
                    nc.scalar.activation(
                        A[:], psv, AF.Identity, bias=bias, scale=scale
                    )
                    A3 = A[:].rearrange("p (n l) -> p n l", n=ng)
                    nc.vector.scalar_tensor_tensor(
                        dsl, A3, 0.1, A3, op0=OP.mult, op1=OP.max,
                    )
                else:
                    # conv4: bf16 z eviction + ||z||^2 partials + fp8 z8
                    # cast (position-major, x16) for the DR distance.
                    dsl = hdst[:, gp * Lout : (gp + ng) * Lout]
                    nc.scalar.activation(
                        dsl, psv, AF.Identity, bias=bias, scale=scale
                    )
                    # z8 cast FIRST on the DVE so the distance regions
                    # (gated on z8) unblock before the square/reduce chain
                    z83 = z8[:].rearrange("p (l n) -> p l n", n=NS)
                    dsl3 = dsl.rearrange("p (n l) -> p n l", n=ng)
                    nc.vector.tensor_scalar_mul(
                        z83[:, 0:LFIN, gp : gp + ng].rearrange("p l n -> p n l"),
                        dsl3,
                        16.0,
                    )
                    zsl = zsq[:, gp * LFIN : (gp + ng) * LFIN]
                    # square on Pool (probe-verified legal) so the DVE tail
                    # only carries the casts + reduces
                    nc.gpsimd.tensor_tensor(zsl, dsl, dsl, op=OP.mult)
                    nc.vector.tensor_reduce(
                        part[:, gp : gp + ng],
                        zsl.rearrange("p (n l) -> p n l", n=ng),
                        axis=mybir.AxisListType.X,
                        op=OP.add,
                    )

            def dist_block(p):
                """d2 for 16 samples in one PSUM bank (partition base 0):
                cn (start) -> 30 fp8-DR position-pair chunks -> zn (stop);
                cn/zn are fp32 matmuls (e0 x cnrow, part x ones)."""
                dp = psD.tile([16, KCENT], F32, tag="d")
                dps[p] = dp
                nc.tensor.matmul(
                    dp[:], bp[:, 136:152], bp[:, 72:136],
                    start=True, stop=False,
                )
                for lp in range(0, 60, 2):
                    lhsT = z84[:, lp : lp + 2, 16 * p : 16 * p + 16]
                    nc.tensor.matmul(
                        dp[:], lhsT, cr3[:, lp : lp + 2, :],
                        start=False, stop=False, perf_mode=DR,
                    )
                nc.tensor.matmul(
                    dp[:], part[:, 16 * p : 16 * p + 16], bp[:, 8:72],
                    start=False, stop=True,
                )

            def q_block(p):
                """q = normalize(1/d2') for 16 samples; DMA out per region."""
                dp = dps[p]
                qn = mpool.tile([16, KCENT], F32, tag=f"qn{p}")
                nc.vector.reciprocal(qn[:], dp[:])
                rs = mpool.tile([16, 1], F32, tag=f"rs{p}")
                nc.vector.tensor_reduce(
                    rs[:], qn[:], axis=mybir.AxisListType.X, op=OP.add
                )
                rr = mpool.tile([16, 1], F32, tag=f"rr{p}")
                nc.vector.reciprocal(rr[:], rs[:])
                nc.vector.tensor_scalar_mul(qn[:], qn[:], rr[:])
                nc.sync.dma_start(q_d.ap()[16 * p : 16 * p + 16, :], qn[:])

            for blk in _schedule():
                if blk[0] == "c":
                    conv_block(blk[1], blk[2])
                elif blk[0] == "d":
                    dist_block(blk[1])
                else:
                    q_block(blk[1])


def _get_program(n_repeat=1):
    if n_repeat not in _BUILt:
        _BUILt[n_repeat] = _build_program(n_repeat)
    return _BUILt[n_repeat]


def _to_f8(a):
    """fp32 -> TRN E4M3 (max 240; clip so OCP e4m3fn bit patterns match)."""
    return np.clip(a, -240.0, 240.0).astype(ml_dtypes.float8_e4m3fn)


def _prep_inputs(x, w1, b1, w2, b2, w3, b3, w4, b4, centers):
    """Host-side prep: fp8 quantization, weight transposes, sharding."""
    ws = [w1, w2, w3, w4]
    bs = [b1, b2, b3, b4]

    const_map = {}
    scales = []
    for i, w in enumerate(ws):
        K, Kp = CFG[i][0], CFG[i][1]
        wf = np.asarray(w, np.float32)  # (O, I, K)
        # per-layer power-of-2 scale-up to ~224 max magnitude (e4m3 headroom)
        mx = float(np.abs(wf).max())
        k = int(np.floor(np.log2(224.0 / mx))) if mx > 0 else 0
        scales.append(2.0 ** (-k))
        wq = wf * (2.0 ** k)
        # (O, I, K) -> (I, Kp, O): lhsT tap k = [:, k*128:(k+1)*128]
        wp = np.zeros((C, Kp, C), np.float32)
        wp[:, :K, :] = wq.transpose(1, 2, 0)
        const_map[f"w{i+1}"] = _to_f8(wp.reshape(C, Kp * C))

    cent = np.asarray(centers, np.float32)
    # cr8[c, l*64 + j] = (-2/16) * centers[j, c*59 + l]; position chunk 59
    # is zero (pairs the z8 pad so the DR distance contracts 60 positions).
    # The 1/16 undoes z8's x16 pre-scale (both powers of 2, exact).
    cr8 = np.zeros((C, 60, KCENT), np.float32)
    cr8[:, :LFIN, :] = (
        (-2.0 / 16.0 * cent).reshape(KCENT, C, LFIN).transpose(1, 2, 0)
    )
    const_map["cr"] = _to_f8(cr8.reshape(C, 60 * KCENT))
    cn = 1.0 + (cent.astype(np.float64) ** 2).sum(axis=1)  # (64,)

    bpk = np.zeros((C, 170), np.float32)
    for i, b in enumerate(bs):
        bpk[:, i] = np.asarray(b, np.float32)
        bpk[:, 4 + i] = scales[i]
    bpk[:, 8:72] = 1.0                      # zn mm rhs (ones)
    bpk[0, 72:136] = cn.astype(np.float32)  # cn mm rhs (row 0 only)
    bpk[0, 136:168] = 1.0                   # cn mm lhsT e0 (row 0 only)
    bpk[:, 168] = 16.0 * scales[3]
    bpk[:, 169] = 16.0 * np.asarray(bs[3], np.float32)
    const_map["bp"] = bpk

    xf = np.asarray(x, np.float32)
    in_maps = []
    for c in range(N_CORES):
        shard = xf[c * NS : (c + 1) * NS]  # (32, 128, 1024)
        xc = _to_f8(np.ascontiguousarray(shard.transpose(1, 0, 2)))  # (128,32,1024)
        in_maps.append({"x": xc, **const_map})
    return in_maps


def _ensure_devices():
    """Absorb wedged-device attach faults with a tiny op before the real run.

    A previous process can leave a NeuronCore wedged
    (NRT_EXEC_UNIT_UNRECOVERABLE); the first attach after a wedge fails and
    triggers a reset that completes within ~60 s.
    """
    import time

    import jax
    import jax.numpy as jnp

    for attempt in range(3):
        try:
            outs = [jax.device_put(jnp.zeros((8,)), d) + 1.0 for d in jax.devices()]
            jax.block_until_ready(outs)
            return
        except Exception:  # noqa: BLE001 - device fault; wait out the reset
            if attempt == 2:
                raise
            time.sleep(60)


def run(trace=False, **inputs):
    """Run the kernel; returns (q_full, BassKernelResults).

    Retries on device-unrecoverable faults (see _ensure_devices).
    """
    import time

    _ensure_devices()
    nc = _get_program()
    in_maps = _prep_inputs(**inputs)
    last_err = None
    for attempt in range(3):
        try:
            res = bass_utils.run_bass_kernel_spmd(
                nc, in_maps, core_ids=list(range(N_CORES)), trace=trace
            )
            break
        except Exception as e:  # noqa: BLE001 - device fault, wait + retry
            last_err = e
            if "UNAVAILABLE" not in str(e) and "unrecoverable" not in str(e).lower():
                raise
            time.sleep(60)
    else:
        raise last_err
    q = np.concatenate([res.results[c]["q"] for c in range(N_CORES)], axis=0)
    return np.ascontiguousarray(q.astype(np.float32)), res


def kernel(**inputs) -> np.ndarray:
    q, _ = run(trace=False, **inputs)
    return q


# revision 56
# speedup vs baseline: 1.3830x; 1.3830x over previous
"""Trainium2 Bass kernel for nn_DEC_26139170963600 (vq_codebook).

Reference computation:
  4x strided conv1d (stride 2, VALID) with LeakyReLU(0.1) between layers,
  flatten -> soft VQ assignment over 64 centers:
      d2 = ||z||^2 + ||c||^2 - 2 z.c
      q  = (1/(1+d2)) row-normalized            (alpha=1 -> exponent is 1)

Sharding: data-parallel over batch N=256 across 8 cores (32 samples/core).
Weights / centers replicated. No cross-device communication.

Per-core kernel design (fp8 DoubleRow conv stack):
  - x and all conv weights quantized host-side to fp8e4 (TRN E4M3, max 240).
    Weights are pre-scaled by a per-layer power of two (up to ~224 max mag)
    to clear e4m3 subnormals; the inverse scale rides the PSUM eviction.
  - conv layer = K/2 tap-pair matmuls in MatmulPerfMode.DoubleRow, which
    virtualizes the PE to a 256-deep contraction (2 fp8 weights/cell):
        out[o, l] += sum_i W[o,:,k+i]^T . h[:, 2l+k+i],  i in {0,1}
    lhsT = (128, 2, 128) tap-pair slice of the packed weights; rhs is the
    contiguous slice h[k : k+2*Lout] rearranged "(l two) -> two l" (the
    stride-2 conv makes tap pairs adjacent in memory). Odd K padded with a
    zero tap (conv1 15->16, conv3 7->8; h2 rows are stride-248-padded so
    the zero tap's read stays in-bounds).
  - PSUM eviction is TWO ops (was 3): ACT A = ps*2^-k + b (PSUM->SBUF
    bf16), then the whole LeakyReLU as ONE DVE scalar_tensor_tensor:
        h' = (A * 0.1) max A -> fp8   (exact lrelu since 0.1 < 1)
    This halves the old DVE+Pool eviction backlog that stalled the PE.
  - HW-verified op constraints (all-8-core probes; do not regress):
      * scalar_tensor_tensor is DVE-only (fails Pool's neuronxcc ISA check)
      * DVE tensor_scalar divide fails the ISA check
      * ACT Prelu hard-faults the device when >=4 cores run it (probe-
        verified in a previous session); the conv4 z8/Square-on-ACT
        variant and 2-sample conv blocks showed the same fault class at
        kernel scale - all avoided here. fp16 matmuls hard-fault too.
  - conv4 evicts bf16 z (no lrelu); distance stays bf16: 59 matmuls
    accumulate -2 z.c into PSUM (32n x 64j); ||z||^2 via per-group DVE
    square+reduce overlapped with conv4, then an fp32 matmul against a
    ones column; 1 + ||c||^2 is a host-precomputed (32,64) fp32 tile.
  - q = reciprocal(1+d2) row-normalized on DVE, DMA out as fp32.
  - PE pre-warm: dummy matmuls during the w1/x DMA lead-in so HAM
    un-throttles (0.65 -> 2.4 GHz) before real conv work arrives.

Measured (8 axon trn2 cores): max rel err ~2.3e-3 vs fp32 reference.
TimelineSim exec time: see test.py (original baseline was 63209 ns).
"""

import os
import sys

import numpy as np
import ml_dtypes

for _p in ("/opt/trn_rl_repo",):
    if _p not in sys.path and os.path.isdir(_p):
        sys.path.insert(0, _p)

import concourse.bacc as bacc  # noqa: E402
import concourse.mybir as mybir  # noqa: E402
import concourse.tile as tile  # noqa: E402
from concourse import bass_utils  # noqa: E402

F8 = mybir.dt.float8e4
HDT = mybir.dt.bfloat16
F32 = mybir.dt.float32
AF = mybir.ActivationFunctionType
OP = mybir.AluOpType
DR = mybir.MatmulPerfMode.DoubleRow

N_CORES = 8
NS = 32          # samples per core
C = 128          # channels
KCENT = 64       # number of centers
LFIN = 59        # final length
D = C * LFIN     # 7552

# (K_real, K_padded, L_in_row_stride, L_out, L_out_row_stride, G samples/mm)
# conv2's output rows carry 1 pad element (248) so conv3's zero tap 7 reads
# in-bounds; the pad is memset once.
CFG = [
    (15, 16, 1024, 505, 505, 1),
    (12, 12, 505, 247, 248, 2),
    (7, 8, 248, 121, 121, 4),
    (4, 4, 121, 59, 59, 8),
]

N_WARM = 29  # PE pre-warm dummy matmuls


def _schedule():
    """Block emission order (identical to the HW-proven baseline weave):
    four 8-sample conv streams s0-s3 (conv1 blocks = 2 samples, conv2 = 4,
    conv3/4 = 8) and two 16-sample distance regions; each stage's eviction
    latency hides under another stream's PE work."""
    seq = [("c", 0, p) for p in range(16)]
    seq += [("c", 1, p) for p in range(8)]
    seq += [("c", 2, p) for p in range(4)]
    seq += [("c", 3, p) for p in range(4)]
    seq += [("d", 0), ("q", 0), ("d", 1), ("q", 1)]
    return seq


_BUILt = {}


def _build_program(n_repeat=1):
    """Build + compile the per-core Bass program (same program on all cores)."""
    nc = bacc.Bacc("TRN2", target_bir_lowering=False, debug=False)

    # ---- DRAM I/O ----
    x_d = nc.dram_tensor("x", (C, NS, 1024), F8, kind="ExternalInput")
    w_d = [
        nc.dram_tensor(f"w{i+1}", (C, CFG[i][1] * C), F8, kind="ExternalInput")
        for i in range(4)
    ]
    # bias/scale pack: cols 0-3 = b1..b4; cols 4-7 = 2^-k1..2^-k4;
    # cols 8-71 = ones (zn mm rhs); cols 72-135 = row0-only 1+||c||^2
    # (cn mm rhs); cols 136-167 = e0 (partition-0 ones, cn mm lhsT)
    bp_d = nc.dram_tensor("bp", (C, 170), F32, kind="ExternalInput")
    # fp8 centers: cr8[c, l*64+j] = (-2/16)*centers[j, c*59+l], chunk 59 = 0
    cr_d = nc.dram_tensor("cr", (C, 60 * KCENT), F8, kind="ExternalInput")
    q_d = nc.dram_tensor("q", (NS, KCENT), F32, kind="ExternalOutput")

    with tile.TileContext(nc) as tc:
        with (
            tc.tile_pool(name="consts", bufs=1) as cpool,
            tc.tile_pool(name="xp", bufs=8) as xpool,
            tc.tile_pool(name="hp", bufs=1) as hpool,
            tc.tile_pool(name="ap", bufs=4) as apool,
            tc.tile_pool(name="tp", bufs=4) as tpool,
            tc.tile_pool(name="small", bufs=1) as mpool,
            tc.tile_pool(name="psA", bufs=3, space="PSUM") as psA,
            tc.tile_pool(name="psD", bufs=2, space="PSUM") as psD,
        ):
            wt = [
                cpool.tile([C, CFG[i][1] * C], F8, tag=f"w{i}", name=f"wt{i}")
                for i in range(4)
            ]
            bp = cpool.tile([C, 170], F32, tag="bp")
            cr = cpool.tile([C, 60 * KCENT], F8, tag="cr")

            for _rep in range(n_repeat):
                _body_once(nc, tc, x_d, q_d, w_d, bp_d, cr_d, wt, bp,
                           cr, xpool, hpool, apool, tpool, mpool,
                           psA, psD, load_consts=(_rep == 0))

    nc.compile()
    return nc


def _body_once(nc, tc, x_d, q_d, w_d, bp_d, cr_d, wt, bp, cr,
               xpool, hpool, apool, tpool, mpool, psA, psD,
               load_consts=True):
            # ---- Two HWDGE rings: x chunks stream on the SP ring while
            # all constants go on the ACT ring, so w1 arrives concurrently
            # with x0 and conv1 starts early ----
            if load_consts:
                nc.scalar.dma_start(wt[0][:], w_d[0].ap())
                nc.scalar.dma_start(bp[:], bp_d.ap())
            xch = []
            for g in range(16):
                t = xpool.tile([C, 2 * 1024], F8, tag="x", name=f"xch{g}")
                src = x_d.ap()[:, 2 * g : 2 * g + 2, :].rearrange("p a b -> p (a b)")
                nc.sync.dma_start(t[:], src)
                xch.append(t)
            if load_consts:
                for i in range(1, 4):
                    nc.scalar.dma_start(wt[i][:], w_d[i].ap())
                nc.scalar.dma_start(cr[:], cr_d.ap())

                # ---- PE pre-warm: HAM un-throttles (0.65 -> 2.4 GHz) after
                # ~3us of sustained activity; burn the w1/x0 DMA lead-in on
                # dummy matmuls over a zeroed scratch so conv1 starts warm ----
                wsrc = tpool.tile([1, 128], HDT, tag="warm", name="warm")
                nc.gpsimd.memset(wsrc[:], 0.0)
                wps = psA.tile([C, 128], F32, tag="ps", name="warmps")
                for _w in range(N_WARM):
                    nc.tensor.matmul(
                        wps[:], wsrc[:], wsrc[:],
                        start=(_w == 0), stop=(_w == N_WARM - 1)
                    )

            # ---- shared tiles for the conv stack / distance tail ----
            h_tiles = []
            for li, (K, Kp, Lin, Lout, Lrow, G) in enumerate(CFG):
                if li == 3:
                    hdst = hpool.tile([C, NS * Lout], HDT, tag=f"h{li}")
                else:
                    hdst = hpool.tile([C, NS * Lrow], F8, tag=f"h{li}")
                    if Lrow > Lout:
                        # zero the per-sample pad so the zero tap's
                        # in-bounds read never multiplies NaN garbage
                        padv = hdst[:].rearrange("p (n l) -> p n l", n=NS)
                        nc.gpsimd.memset(padv[:, :, Lout:Lrow], 0.0)
                h_tiles.append(hdst)
            zsq = hpool.tile([C, NS * LFIN], F32, tag="zsq", name="zsq")
            part = mpool.tile([C, NS], F32, tag="part", name="part")
            z8 = hpool.tile([C, 60 * NS], F8, tag="z8", name="z8")
            nc.gpsimd.memset(z8[:, LFIN * NS : 60 * NS], 0.0)
            z84 = z8[:].rearrange("p (l n) -> p l n", n=NS)
            cr3 = cr[:].rearrange("p (l j) -> p l j", j=KCENT)
            dps = [None] * 2

            def conv_block(li, pr):
                """One PSUM block: conv1-3 = two G-sample halves sharing a
                2-bank tile; conv4 = one 8-sample group on half a tile."""
                K, Kp, Lin, Lout, Lrow, G = CFG[li]
                hdst = h_tiles[li]
                hdst3 = hdst[:].rearrange("p (n l) -> p n l", n=NS)
                if li > 0:
                    hsrc3 = h_tiles[li - 1][:].rearrange("p (n l) -> p n l", n=NS)
                nhalf = 1 if li == 3 else 2
                gp = pr * nhalf * G
                ps = psA.tile([C, 1024], F32, tag="ps")
                for half in range(nhalf):
                    g0 = gp + half * G
                    pslice = ps[:, half * 512 : half * 512 + G * Lout]
                    for kp in range(0, Kp, 2):
                        lhsT = wt[li][:, kp * C : (kp + 2) * C].rearrange(
                            "p (two o) -> p two o", two=2
                        )
                        if li == 0:
                            x3 = xch[g0 // 2][:].rearrange("p (a b) -> p a b", a=2)
                            rhs = x3[
                                :, g0 % 2 : g0 % 2 + 1, kp : kp + 2 * Lout
                            ].rearrange("p n (l two) -> p two n l", two=2)
                        else:
                            rhs = hsrc3[
                                :, g0 : g0 + G, kp : kp + 2 * Lout
                            ].rearrange("p n (l two) -> p two n l", two=2)
                        nc.tensor.matmul(
                            pslice, lhsT, rhs,
                            start=(kp == 0), stop=(kp == Kp - 2),
                            perf_mode=DR,
                        )
                bias = bp[:, li : li + 1]
                scale = bp[:, 4 + li : 5 + li]
                ng = nhalf * G
                E = ng * Lout
                psv = (
                    ps[:, 0 : G * Lout]
                    if li == 3
                    else ps[:].rearrange("p (g l) -> p g l", g=2)[:, :, 0 : G * Lout]
                )
                if li < 3:
                    # eviction v2: ACT A = ps*2^-k + b, then lrelu in ONE
                    # DVE op: h = (0.1*A) max A -> fp8
                    dsl = hdst3[:, gp : gp + ng, 0:Lout]
                    A = apool.tile([C, E], HDT, tag="A")
                    nc.scalar.activation(
                        A[:], psv, AF.Identity, bias=bias, scale=scale
                    )
                    A3 = A[:].rearrange("p (n l) -> p n l", n=ng)
                    nc.vector.scalar_tensor_tensor(
                        dsl, A3, 0.1, A3, op0=OP.mult, op1=OP.max,
                    )
                else:
                    # conv4: bf16 z eviction + ||z||^2 partials + fp8 z8
                    # cast (position-major, x16) for the DR distance.
                    dsl = hdst[:, gp * Lout : (gp + ng) * Lout]
                    nc.scalar.activation(
                        dsl, psv, AF.Identity, bias=bias, scale=scale
                    )
                    # z8 cast FIRST on the DVE so the distance regions
                    # (gated on z8) unblock before the square/reduce chain
                    z83 = z8[:].rearrange("p (l n) -> p l n", n=NS)
                    dsl3 = dsl.rearrange("p (n l) -> p n l", n=ng)
                    nc.vector.tensor_scalar_mul(
                        z83[:, 0:LFIN, gp : gp + ng].rearrange("p l n -> p n l"),
                        dsl3,
                        16.0,
                    )
                    zsl = zsq[:, gp * LFIN : (gp + ng) * LFIN]
                    # square on Pool (probe-verified legal) so the DVE tail
                    # only carries the casts + reduces
                    nc.gpsimd.tensor_tensor(zsl, dsl, dsl, op=OP.mult)
                    nc.vector.tensor_reduce(
                        part[:, gp : gp + ng],
                        zsl.rearrange("p (n l) -> p n l", n=ng),
                        axis=mybir.AxisListType.X,
                        op=OP.add,
                    )

            def dist_block(p):
                """d2 for 16 samples in one PSUM bank (partition base 0):
                cn (start) -> 30 fp8-DR position-pair chunks -> zn (stop);
                cn/zn are fp32 matmuls (e0 x cnrow, part x ones)."""
                dp = psD.tile([16, KCENT], F32, tag="d")
                dps[p] = dp
                nc.tensor.matmul(
                    dp[:], bp[:, 136:152], bp[:, 72:136],
                    start=True, stop=False,
                )
                for lp in range(0, 60, 2):
                    lhsT = z84[:, lp : lp + 2, 16 * p : 16 * p + 16]
                    nc.tensor.matmul(
                        dp[:], lhsT, cr3[:, lp : lp + 2, :],
                        start=False, stop=False, perf_mode=DR,
                    )
                nc.tensor.matmul(
                    dp[:], part[:, 16 * p : 16 * p + 16], bp[:, 8:72],
                    start=False, stop=True,
                )

            def q_block(p):
                """q = normalize(1/d2') for 16 samples; DMA out per region."""
                dp = dps[p]
                qn = mpool.tile([16, KCENT], F32, tag=f"qn{p}")
                nc.vector.reciprocal(qn[:], dp[:])
                rs = mpool.tile([16, 1], F32, tag=f"rs{p}")
                nc.vector.tensor_reduce(
                    rs[:], qn[:], axis=mybir.AxisListType.X, op=OP.add
                )
                rr = mpool.tile([16, 1], F32, tag=f"rr{p}")
                nc.vector.reciprocal(rr[:], rs[:])
                nc.vector.tensor_scalar_mul(qn[:], qn[:], rr[:])
                nc.sync.dma_start(q_d.ap()[16 * p : 16 * p + 16, :], qn[:])

            for blk in _schedule():
                if blk[0] == "c":
                    conv_block(blk[1], blk[2])
                elif blk[0] == "d":
                    dist_block(blk[1])
                else:
                    q_block(blk[1])


def _get_program(n_repeat=1):
    if n_repeat not in _BUILt:
        _BUILt[n_repeat] = _build_program(n_repeat)
    return _BUILt[n_repeat]


def _to_f8(a):
    """fp32 -> TRN E4M3 (max 240; clip so OCP e4m3fn bit patterns match)."""
    return np.clip(a, -240.0, 240.0).astype(ml_dtypes.float8_e4m3fn)


def _prep_inputs(x, w1, b1, w2, b2, w3, b3, w4, b4, centers):
    """Host-side prep: fp8 quantization, weight transposes, sharding."""
    ws = [w1, w2, w3, w4]
    bs = [b1, b2, b3, b4]

    const_map = {}
    scales = []
    for i, w in enumerate(ws):
        K, Kp = CFG[i][0], CFG[i][1]
        wf = np.asarray(w, np.float32)  # (O, I, K)
        # per-layer power-of-2 scale-up to ~224 max magnitude (e4m3 headroom)
        mx = float(np.abs(wf).max())
        k = int(np.floor(np.log2(224.0 / mx))) if mx > 0 else 0
        scales.append(2.0 ** (-k))
        wq = wf * (2.0 ** k)
        # (O, I, K) -> (I, Kp, O): lhsT tap k = [:, k*128:(k+1)*128]
        wp = np.zeros((C, Kp, C), np.float32)
        wp[:, :K, :] = wq.transpose(1, 2, 0)
        const_map[f"w{i+1}"] = _to_f8(wp.reshape(C, Kp * C))

    cent = np.asarray(centers, np.float32)
    # cr8[c, l*64 + j] = (-2/16) * centers[j, c*59 + l]; position chunk 59
    # is zero (pairs the z8 pad so the DR distance contracts 60 positions).
    # The 1/16 undoes z8's x16 pre-scale (both powers of 2, exact).
    cr8 = np.zeros((C, 60, KCENT), np.float32)
    cr8[:, :LFIN, :] = (
        (-2.0 / 16.0 * cent).reshape(KCENT, C, LFIN).transpose(1, 2, 0)
    )
    const_map["cr"] = _to_f8(cr8.reshape(C, 60 * KCENT))
    cn = 1.0 + (cent.astype(np.float64) ** 2).sum(axis=1)  # (64,)

    bpk = np.zeros((C, 170), np.float32)
    for i, b in enumerate(bs):
        bpk[:, i] = np.asarray(b, np.float32)
        bpk[:, 4 + i] = scales[i]
    bpk[:, 8:72] = 1.0                      # zn mm rhs (ones)
    bpk[0, 72:136] = cn.astype(np.float32)  # cn mm rhs (row 0 only)
    bpk[0, 136:168] = 1.0                   # cn mm lhsT e0 (row 0 only)
    bpk[:, 168] = 16.0 * scales[3]
    bpk[:, 169] = 16.0 * np.asarray(bs[3], np.float32)
    const_map["bp"] = bpk

    xf = np.asarray(x, np.float32)
    in_maps = []
    for c in range(N_CORES):
        shard = xf[c * NS : (c + 1) * NS]  # (32, 128, 1024)
        xc = _to_f8(np.ascontiguousarray(shard.transpose(1, 0, 2)))  # (128,32,1024)
        in_maps.append({"x": xc, **const_map})
    return in_maps


def _ensure_devices():
    """Absorb wedged-device attach faults with a tiny op before the real run.

    A previous process can leave a NeuronCore wedged
    (NRT_EXEC_UNIT_UNRECOVERABLE); the first attach after a wedge fails and
    triggers a reset that completes within ~60 s.
    """
    import time

    import jax
    import jax.numpy as jnp

    for attempt in range(3):
        try:
            outs = [jax.device_put(jnp.zeros((8,)), d) + 1.0 for d in jax.devices()]
            jax.block_until_ready(outs)
            return
        except Exception:  # noqa: BLE001 - device fault; wait out the reset
            if attempt == 2:
                raise
            time.sleep(60)


def run(trace=False, **inputs):
    """Run the kernel; returns (q_full, BassKernelResults).

    Retries on device-unrecoverable faults (see _ensure_devices).
    """
    import time

    _ensure_devices()
    nc = _get_program()
    in_maps = _prep_inputs(**inputs)
    last_err = None
    for attempt in range(3):
        try:
            res = bass_utils.run_bass_kernel_spmd(
                nc, in_maps, core_ids=list(range(N_CORES)), trace=trace
            )
            break
        except Exception as e:  # noqa: BLE001 - device fault, wait + retry
            last_err = e
            if "UNAVAILABLE" not in str(e) and "unrecoverable" not in str(e).lower():
                raise
            time.sleep(60)
    else:
        raise last_err
    q = np.concatenate([res.results[c]["q"] for c in range(N_CORES)], axis=0)
    return np.ascontiguousarray(q.astype(np.float32)), res


def kernel(**inputs) -> np.ndarray:
    q, _ = run(trace=False, **inputs)
    return q
